# revision 1
# baseline (speedup 1.0000x reference)
"""Trainium2 Bass kernel for nn_BDH_1726576853700 (sparse_attention).

3-layer sparse-attention net: B=1, T=1024, D=256, NH=4, N=8192, VOCAB=256.

Sharding over 8 NeuronCores: device d -> (head h=d//2, half=d%2) — each device
owns a 4096-wide slice of one head's sparse latent dim.  Within the slice the
latent index is permuted evens-first so the RoPE pair partner sits exactly 2048
rows away (tile i <-> tile i+16), turning the pair rotation into whole-tile
elementwise ops.  Per layer:
  - x_sparse^T = relu(enc_w^T @ x^T)   (local)
  - qr = rope(x_sparse)                (local, host-precomputed cos/sin tables)
  - S_partial = qr^T qr (local n contraction), strictly-causal masked
  - ykv_partial = S_masked^T @ x ; pair AllReduce (the two halves of one head)
  - ykv_ln = layernorm(ykv); y_sparse^T = relu(encv_w^T @ ykv_ln^T) (local)
  - ymlp^T_partial = dec^T-contracted with (x_sparse * y_sparse)    (local)
  - 8-way AllReduce(ymlp); x = ln(x + ln(ymlp)) (replicated)
Collectives run in fp16 (halves wire bytes); matmuls run in fp16 with fp32
PSUM accumulation; the residual stream x is kept in fp32 on-chip.

PSUM discipline: every accumulation group owns its bank(s) exclusively —
`start=True` clears has_written bits for the WHOLE bank, so two interleaved
groups must never share a bank.
"""

import math
import sys

for _p in ("/opt/trn_rl_repo",):
    if _p not in sys.path:
        sys.path.insert(0, _p)

import numpy as np

import concourse.bass as bass
import concourse.mybir as mybir
import concourse.tile as tile
from concourse import bacc, bass_utils
from concourse.masks import make_identity

# ---- problem constants (hardcoded per contract) ----
B, T, D, NH, N = 1, 1024, 256, 4, 8192
VOCAB = 256
N_LAYER = 3
EPS = 1e-5
TWO_PI = 2.0 * math.pi
N_CORES = 8
NLOC = N // 2          # latent columns per device: 4096
P = 128
NT = T // P            # 8 t-tiles
KD = D // P            # 2 d-tiles
NM = NLOC // P         # 32 n-tiles per device
NPAIR = NM // 2        # 16 rope pairs
HDT = mybir.dt.float16     # on-chip activation dtype
F32 = mybir.dt.float32
YKV_SCALE = 1.0 / 256.0    # keeps ykv in fp16 range; LN downstream is
                           # scale-invariant so the result is unchanged

_CACHE = {}


def _build_program(dbg=False, use_collectives=True, rope_gpsimd=False, skip_scores=False, skip_proj=False, n_layers=N_LAYER):
    def emit_allreduce(nc, groups, ins, outs):
        if use_collectives:
            nc.gpsimd.collective_compute(
                "AllReduce", mybir.AluOpType.add, replica_groups=groups,
                ins=ins, outs=outs)
        else:
            # timing/sim variant: replace the collective with a plain copy
            nc.sync.dma_start(outs[0], ins[0])
    nc = bacc.Bacc("TRN2", target_bir_lowering=False, debug=False,
                   num_devices=N_CORES)
    dbg_tensors = {}
    if dbg:
        for nm, shape, dt in [
            ("dbg_x0ln", [T, D], F32),
            ("dbg_xsp", [NM * P, T], HDT),
            ("dbg_qr", [NM * P, T], HDT),
            ("dbg_ykvpre", [T, D], HDT),
            ("dbg_ykvpost", [T, D], HDT),
            ("dbg_ykvT", [D, T], HDT),
            ("dbg_ymlppre", [D, T], HDT),
            ("dbg_ymlppost", [D, T], HDT),
            ("dbg_x1", [T, D], F32),
        ]:
            dbg_tensors[nm] = nc.dram_tensor(nm, shape, dt,
                                             kind="ExternalOutput")

    x0_d = nc.dram_tensor("x0", [T, D], F32, kind="ExternalInput")
    encw_d = nc.dram_tensor("encw", [D, NLOC], HDT, kind="ExternalInput")
    encvw_d = nc.dram_tensor("encvw", [D, NLOC], HDT, kind="ExternalInput")
    decw_d = nc.dram_tensor("decw", [NLOC, D], HDT, kind="ExternalInput")
    ct_d = nc.dram_tensor("ct", [NLOC // 2, T], HDT, kind="ExternalInput")
    st_d = nc.dram_tensor("st", [NLOC // 2, T], HDT, kind="ExternalInput")
    lmh_d = nc.dram_tensor("lmh", [D, VOCAB], HDT, kind="ExternalInput")
    umask_d = nc.dram_tensor("umask", [P, P], F32, kind="ExternalInput")
    logits_d = nc.dram_tensor("logits", [T, VOCAB], F32, kind="ExternalOutput")

    PAIR_GROUPS = [[0, 1], [2, 3], [4, 5], [6, 7]]
    ALL_GROUP = [list(range(N_CORES))]

    with tile.TileContext(nc) as tc:
        persist = tc.alloc_tile_pool(name="persist", bufs=1)
        dram = tc.alloc_tile_pool(name="dram", bufs=1, space="DRAM")

        # persistent SBUF state
        x_sp = persist.tile([P, NM, T], HDT)        # x_sparse^T tiles
        qr = persist.tile([P, NM, T], HDT)          # roped x_sparse^T
        x_f32 = persist.tile([P, NT, D], F32)       # residual stream (natural)
        x_h = persist.tile([P, NT, D], HDT)         # x natural fp16
        xT_h = persist.tile([P, KD, T], HDT)        # x^T fp16
        ykvT_h = persist.tile([P, KD, T], HDT)      # ykv_ln^T fp16
        lmh_sb = persist.tile([P, KD, VOCAB], HDT)
        umask_sb = persist.tile([P, P], F32)
        ident = persist.tile([P, P], HDT)

        eps_sb = persist.tile([P, 1], F32)
        nc.vector.memset(eps_sb[:], float(EPS))
        nc.sync.dma_start(umask_sb[:], umask_d.ap())
        make_identity(nc, ident[:])
        for k in range(KD):
            nc.sync.dma_start(lmh_sb[:, k, :], lmh_d.ap()[k * P:(k + 1) * P, :])

        # streaming / working pools (live across the whole kernel)
        wenc = tc.alloc_tile_pool(name="wenc", bufs=3)
        wdec = tc.alloc_tile_pool(name="wdec", bufs=4)
        csp = tc.alloc_tile_pool(name="csp", bufs=2)
        ropep = tc.alloc_tile_pool(name="ropep", bufs=2)
        schp = tc.alloc_tile_pool(name="schp", bufs=2)
        sdp = tc.alloc_tile_pool(name="sdp", bufs=2)
        yxp = tc.alloc_tile_pool(name="yxp", bufs=2)
        arp = tc.alloc_tile_pool(name="arp", bufs=1)
        lnp = tc.alloc_tile_pool(name="lnp", bufs=2)
        statp = tc.alloc_tile_pool(name="statp", bufs=4)

        def layer_norm(src_ap, out_ap):
            """LayerNorm over the free dim (size D) of a [P, D] tile."""
            stats = statp.tile([P, 6], F32, name="ln_stats")
            mv = statp.tile([P, 2], F32, name="ln_mv")
            rstd = statp.tile([P, 1], F32, name="ln_rstd")
            nc.vector.bn_stats(out=stats[:], in_=src_ap)
            nc.vector.bn_aggr(out=mv[:], in_=stats[:])
            nc.scalar.activation(out=rstd[:], in_=mv[:, 1:2],
                                 func=mybir.ActivationFunctionType.Sqrt,
                                 bias=eps_sb[:])
            nc.vector.reciprocal(out=rstd[:], in_=rstd[:])
            nc.vector.tensor_scalar(out=out_ap, in0=src_ap,
                                    scalar1=mv[:, 0:1], scalar2=rstd[:],
                                    op0=mybir.AluOpType.subtract,
                                    op1=mybir.AluOpType.mult)

        def transpose_into(dst_ap, src_ap, pst_pool):
            """PE-transpose a [P, P] fp16 SBUF block into dst (via PSUM)."""
            pst = pst_pool.tile([P, P], HDT, name="pst")
            nc.tensor.transpose(pst[:], src_ap, ident[:])
            nc.vector.tensor_copy(out=dst_ap, in_=pst[:])

        def set_x_from(j, src_f32_ap, pst_pool):
            """Write x_f32/x_h/xT_h for t-tile j from a normalized f32 tile."""
            if src_f32_ap is not x_f32:
                nc.vector.tensor_copy(out=x_f32[:, j, :], in_=src_f32_ap)
            nc.scalar.copy(out=x_h[:, j, :], in_=x_f32[:, j, :])
            for k in range(KD):
                transpose_into(xT_h[:, k, j * P:(j + 1) * P],
                               x_h[:, j, k * P:(k + 1) * P], pst_pool)

        # ---- initial x = ln(embed[idx]) (gather done on host into x0) ----
        with tc.tile_pool(name="ps_init", bufs=2, space="PSUM") as ps_init:
            for j in range(NT):
                x0t = lnp.tile([P, D], F32, name="x0t")
                nc.sync.dma_start(x0t[:], x0_d.ap()[j * P:(j + 1) * P, :])
                layer_norm(x0t[:], x_f32[:, j, :])
                set_x_from(j, x_f32, ps_init)
        if dbg:
            nc.sync.dma_start(
                dbg_tensors["dbg_x0ln"].ap().rearrange("(j p) d -> p j d", p=P),
                x_f32[:])

        # ---- layers ----
        for layer in range(n_layers):
            # Phase A: x_sparse^T = relu(enc^T x^T), then rope -> qr
            with tc.tile_pool(name=f"psA_{layer}", bufs=2,
                              space="PSUM") as psA:
                for m in range(NM):
                    ps = psA.tile([P, T], F32, name="psA")
                    et = wenc.tile([P, KD, P], HDT, name="enc_t")
                    nc.sync.dma_start(
                        et[:],
                        encw_d.ap().rearrange("(k p) n -> p k n", p=P)[
                            :, :, m * P:(m + 1) * P])
                    for c in range(2):
                        for k in range(1 if skip_proj else KD):
                            nc.tensor.matmul(
                                ps[:, c * 512:(c + 1) * 512],
                                lhsT=et[:, k, :],
                                rhs=xT_h[:, k, c * 512:(c + 1) * 512],
                                start=(k == 0),
                                stop=(k == (0 if skip_proj else KD - 1)))
                    nc.scalar.activation(
                        out=x_sp[:, m, :], in_=ps[:],
                        func=mybir.ActivationFunctionType.Relu)

                for i in range(NPAIR):
                    ctt = csp.tile([P, T], HDT, name="ctt")
                    stt = csp.tile([P, T], HDT, name="stt")
                    nc.sync.dma_start(ctt[:], ct_d.ap()[i * P:(i + 1) * P, :])
                    nc.sync.dma_start(stt[:], st_d.ap()[i * P:(i + 1) * P, :])
                    xe = x_sp[:, i, :]
                    xo = x_sp[:, i + NPAIR, :]
                    t1 = ropep.tile([P, T], HDT, name="rope_t1")
                    t2 = ropep.tile([P, T], HDT, name="rope_t2")
                    eng2 = nc.gpsimd if rope_gpsimd else nc.vector
                    nc.vector.tensor_mul(t1[:], xe, ctt[:])
                    eng2.tensor_mul(t2[:], xo, stt[:])
                    nc.vector.tensor_sub(qr[:, i, :], t1[:], t2[:])
                    t3 = ropep.tile([P, T], HDT, name="rope_t1")
                    t4 = ropep.tile([P, T], HDT, name="rope_t2")
                    nc.vector.tensor_mul(t3[:], xo, ctt[:])
                    eng2.tensor_mul(t4[:], xe, stt[:])
                    nc.vector.tensor_add(qr[:, i + NPAIR, :], t3[:], t4[:])

            if dbg and layer == 0:
                nc.sync.dma_start(
                    dbg_tensors["dbg_xsp"].ap().rearrange(
                        "(m p) t -> p m t", p=P), x_sp[:])
                nc.sync.dma_start(
                    dbg_tensors["dbg_qr"].ap().rearrange(
                        "(m p) t -> p m t", p=P), qr[:])

            # Phase B: S partial + causal mask + ykv partial accumulation.
            # c-major passes so the 4 live ykv accumulators each own a full
            # PSUM bank (plus 2 rotating banks for S chunks).
            ykv_pre = arp.tile([P, NT, D], HDT, name="ykv_pre")
            for c in range(2):
                with tc.tile_pool(name=f"psS_{layer}_{c}", bufs=3,
                                  space="PSUM") as psS, \
                     tc.tile_pool(name=f"psY_{layer}_{c}", bufs=1,
                                  space="PSUM") as psY:
                    ykv_ps = [psY.tile([P, D], F32, name=f"ykv_ps{j}",
                                       tag=f"ykv_ps{j}")
                              for j in range(4 * c, 4 * c + 4)]
                    for i in range(4 * c + 4):
                        # causal tiling: only columns t >= i*P are needed
                        base = max(c * 512, i * P)
                        width = (c + 1) * 512 - base
                        ps = psS.tile([P, 512], F32, name="psS")
                        for k in range(1 if skip_scores else NM):
                            nc.tensor.matmul(
                                ps[:, :width],
                                lhsT=qr[:, k, i * P:(i + 1) * P],
                                rhs=qr[:, k, base:base + width],
                                start=(k == 0),
                                stop=(k == (0 if skip_scores else NM - 1)))
                        sc = schp.tile([P, 512], HDT, name="schunk")
                        if i % 2 == 0:
                            nc.scalar.copy(out=sc[:, :width],
                                           in_=ps[:, :width])
                        else:
                            nc.vector.tensor_copy(out=sc[:, :width],
                                                  in_=ps[:, :width])
                        sd = None
                        if c == i // 4:
                            dcol = i * P - base
                            sd = sdp.tile([P, P], HDT, name="sdiag")
                            nc.vector.tensor_mul(sd[:],
                                                 ps[:, dcol:dcol + P],
                                                 umask_sb[:])
                        for j in range(max(4 * c, i), 4 * c + 4):
                            lhsT = sd[:] if j == i else \
                                sc[:, j * P - base:(j + 1) * P - base]
                            nc.tensor.matmul(
                                ykv_ps[j - 4 * c][:], lhsT=lhsT,
                                rhs=x_h[:, i, :],
                                start=(i == 0), stop=(i == j))
                    for j in range(4 * c, 4 * c + 4):
                        nc.scalar.mul(out=ykv_pre[:, j, :],
                                      in_=ykv_ps[j - 4 * c][:],
                                      mul=YKV_SCALE)

            if dbg and layer == 0:
                nc.sync.dma_start(
                    dbg_tensors["dbg_ykvpre"].ap().rearrange(
                        "(j p) d -> p j d", p=P), ykv_pre[:])

            # Phase C: pair AllReduce of ykv, layernorm, transpose
            ar_in = dram.tile([T, D], HDT, name=f"arin_{layer}",
                              tag=f"arin_{layer}")
            ar_out = dram.tile([T, D], HDT, name=f"arout_{layer}",
                               tag=f"arout_{layer}")
            nc.sync.dma_start(
                ar_in.rearrange("(j p) d -> p j d", p=P), ykv_pre[:])
            emit_allreduce(nc, PAIR_GROUPS, [ar_in.opt()], [ar_out.opt()])
            ykv_post = arp.tile([P, NT, D], HDT, name="ykv_post")
            nc.sync.dma_start(
                ykv_post[:], ar_out.rearrange("(j p) d -> p j d", p=P))
            with tc.tile_pool(name=f"psT_{layer}", bufs=2,
                              space="PSUM") as psT:
                for j in range(NT):
                    yl = lnp.tile([P, D], HDT, name="ykv_ln")
                    layer_norm(ykv_post[:, j, :], yl[:])
                    for k in range(KD):
                        transpose_into(ykvT_h[:, k, j * P:(j + 1) * P],
                                       yl[:, k * P:(k + 1) * P], psT)

            if dbg and layer == 0:
                nc.sync.dma_start(
                    dbg_tensors["dbg_ykvpost"].ap().rearrange(
                        "(j p) d -> p j d", p=P), ykv_post[:])
                nc.sync.dma_start(
                    dbg_tensors["dbg_ykvT"].ap().rearrange(
                        "(k p) t -> p k t", p=P), ykvT_h[:])

            # Phase D: y_sparse^T = relu(encv^T ykv_ln^T); xy = x_sp * y_sp;
            # ymlp^T accumulated transposed: lhsT = decoder tile, rhs = xy.
            # ymlp^T psum tiles span 2 banks each with exactly one
            # accumulation group per bank.
            ymlpT_pre = arp.tile([P, KD, T], HDT, name="ymlpT_pre")
            with tc.tile_pool(name=f"psD_{layer}", bufs=2,
                              space="PSUM") as psD, \
                 tc.tile_pool(name=f"psM_{layer}", bufs=1,
                              space="PSUM") as psM:
                ymlpT_ps = [psM.tile([P, T], F32, name=f"ymlpT_ps{k}",
                                     tag=f"ymlpT_ps{k}") for k in range(KD)]
                for m in range(NM):
                    ps = psD.tile([P, T], F32, name="psD")
                    et = wenc.tile([P, KD, P], HDT, name="encv_t")
                    nc.sync.dma_start(
                        et[:],
                        encvw_d.ap().rearrange("(k p) n -> p k n", p=P)[
                            :, :, m * P:(m + 1) * P])
                    for c in range(2):
                        for k in range(KD):
                            nc.tensor.matmul(
                                ps[:, c * 512:(c + 1) * 512],
                                lhsT=et[:, k, :],
                                rhs=ykvT_h[:, k, c * 512:(c + 1) * 512],
                                start=(k == 0), stop=(k == KD - 1))
                    ysp = yxp.tile([P, T], HDT, name="ysp")
                    nc.scalar.activation(
                        out=ysp[:], in_=ps[:],
                        func=mybir.ActivationFunctionType.Relu)
                    xy = yxp.tile([P, T], HDT, name="xy")
                    nc.vector.tensor_mul(xy[:], x_sp[:, m, :], ysp[:])
                    dm = wdec.tile([P, D], HDT, name="dec_t")
                    nc.sync.dma_start(dm[:],
                                      decw_d.ap()[m * P:(m + 1) * P, :])
                    for k in range(KD):
                        for c in range(2):
                            nc.tensor.matmul(
                                ymlpT_ps[k][:, c * 512:(c + 1) * 512],
                                lhsT=dm[:, k * P:(k + 1) * P],
                                rhs=xy[:, c * 512:(c + 1) * 512],
                                start=(m == 0), stop=(m == NM - 1))
                for k in range(KD):
                    nc.scalar.copy(out=ymlpT_pre[:, k, :],
                                   in_=ymlpT_ps[k][:])

            if dbg and layer == 0:
                nc.sync.dma_start(
                    dbg_tensors["dbg_ymlppre"].ap().rearrange(
                        "(k p) t -> p k t", p=P), ymlpT_pre[:])

            # Phase E: 8-way AllReduce of ymlp^T; x = ln(x + ln(ymlp))
            ar2_in = dram.tile([D, T], HDT, name=f"ar2in_{layer}",
                               tag=f"ar2in_{layer}")
            ar2_out = dram.tile([D, T], HDT, name=f"ar2out_{layer}",
                                tag=f"ar2out_{layer}", addr_space="Shared")
            nc.sync.dma_start(
                ar2_in.rearrange("(k p) t -> p k t", p=P), ymlpT_pre[:])
            emit_allreduce(nc, ALL_GROUP, [ar2_in.opt()], [ar2_out.opt()])
            ymlpT_post = arp.tile([P, KD, T], HDT, name="ymlpT_post")
            nc.sync.dma_start(
                ymlpT_post[:], ar2_out.rearrange("(k p) t -> p k t", p=P))
            if dbg and layer == 0:
                nc.sync.dma_start(
                    dbg_tensors["dbg_ymlppost"].ap().rearrange(
                        "(k p) t -> p k t", p=P), ymlpT_post[:])
            with tc.tile_pool(name=f"psE_{layer}", bufs=2,
                              space="PSUM") as psE:
                for j in range(NT):
                    ymt = lnp.tile([P, D], HDT, name="ymt")
                    for k in range(KD):
                        transpose_into(ymt[:, k * P:(k + 1) * P],
                                       ymlpT_post[:, k, j * P:(j + 1) * P],
                                       psE)
                    u = lnp.tile([P, D], F32, name="u_ln")
                    layer_norm(ymt[:], u[:])
                    xn = lnp.tile([P, D], F32, name="xn")
                    nc.vector.tensor_add(xn[:], x_f32[:, j, :], u[:])
                    layer_norm(xn[:], x_f32[:, j, :])
                    set_x_from(j, x_f32, psE)
            if dbg and layer == 0:
                nc.sync.dma_start(
                    dbg_tensors["dbg_x1"].ap().rearrange(
                        "(j p) d -> p j d", p=P), x_f32[:])

        # ---- logits = x @ lm_head ----
        with tc.tile_pool(name="psL", bufs=2, space="PSUM") as psL:
            for j in range(NT):
                ps = psL.tile([P, VOCAB], F32, name="psLt")
                for k in range(KD):
                    nc.tensor.matmul(ps[:],
                                     lhsT=xT_h[:, k, j * P:(j + 1) * P],
                                     rhs=lmh_sb[:, k, :],
                                     start=(k == 0), stop=(k == KD - 1))
                lg = lnp.tile([P, VOCAB], F32, name="lgt")
                nc.scalar.copy(out=lg[:], in_=ps[:])
                nc.sync.dma_start(logits_d.ap()[j * P:(j + 1) * P, :], lg[:])

        for _pool in (statp, lnp, arp, yxp, sdp, schp, ropep, csp,
                      wdec, wenc, dram, persist):
            _pool.release()

    nc.compile()
    return nc


def _host_inputs(idx, embed, encoder, encoder_v, decoder, lm_head):
    """Build the 8 per-core input maps (host-side sharding)."""
    f16 = np.float16
    idx = np.asarray(idx).reshape(-1).astype(np.int64)
    embed = np.asarray(embed, np.float32)
    enc = np.asarray(encoder, np.float32)
    encv = np.asarray(encoder_v, np.float32)
    dec = np.asarray(decoder, np.float32)
    lmh = np.asarray(lm_head, np.float32)

    x0 = embed[idx]  # [T, D] gather on host (pure indexing)

    # freqs exactly as the reference computes them (fp32)
    t = np.arange(0, N, dtype=np.float32)
    q = np.floor(t / 2.0) * 2.0
    freqs = (1.0 / ((2.0 ** 16) ** (q / N)) / TWO_PI).astype(np.float32)
    tvec = np.arange(T, dtype=np.float32)

    umask = (np.arange(P)[:, None] < np.arange(P)[None, :]).astype(np.float32)

    in_maps = []
    for d in range(N_CORES):
        h, half = d // 2, d % 2
        perm = np.concatenate([np.arange(0, NLOC, 2),
                               np.arange(1, NLOC, 2)]) + half * NLOC
        f_loc = freqs[perm[:NLOC // 2]]
        ph = (tvec[None, :] * f_loc[:, None]).astype(np.float32) % 1.0
        in_maps.append({
            "x0": np.ascontiguousarray(x0, np.float32),
            "encw": np.ascontiguousarray(enc[h][:, perm], f16),
            "encvw": np.ascontiguousarray(encv[h][:, perm], f16),
            "decw": np.ascontiguousarray(dec[h * N + perm, :], f16),
            "ct": np.ascontiguousarray(np.cos(TWO_PI * ph), f16),
            "st": np.ascontiguousarray(np.sin(TWO_PI * ph), f16),
            "lmh": np.ascontiguousarray(lmh, f16),
            "umask": umask,
        })
    return in_maps


def kernel(idx, embed, encoder, encoder_v, decoder, lm_head,
           _trace=False, _tmpdir=None):
    if "nc" not in _CACHE:
        _CACHE["nc"] = _build_program()
    nc = _CACHE["nc"]
    in_maps = _host_inputs(idx, embed, encoder, encoder_v, decoder, lm_head)
    res = bass_utils.run_bass_kernel_spmd(
        nc, in_maps, core_ids=list(range(N_CORES)),
        trace=_trace, tmpdir=_tmpdir)
    _CACHE["last_results"] = res
    logits = res.results[0]["logits"].astype(np.float32).reshape(B, T, VOCAB)
    return logits



# revision 46
# speedup vs baseline: 1.1904x; 1.1904x over previous
"""Trainium2 Bass kernel for nn_BDH_1726576853700 (sparse_attention).

3-layer sparse-attention net: B=1, T=1024, D=256, NH=4, N=8192, VOCAB=256.

Sharding over 8 NeuronCores: device d -> (head h=d//2, half=d%2) — each device
owns a 4096-wide slice of one head's sparse latent dim, permuted evens-first so
the RoPE pair partner of latent tile i is tile i+16.

v2 rewrite (vs the phase-serial baseline):
  - Phase A emits m-tiles in pair-completing order (0,16,1,17,...) with the
    relu split across ACT/DVE/Pool and rope emitted per-pair, so rope overlaps
    the projection matmuls.
  - Scores are computed in a two-pass K-split: group G1 (6-7 chunks resident
    in PSUM) accumulates over the early-roped k-tiles while late pairs are
    still roping, then finishes with the late k-tiles; G2 chunks + the ykv
    contraction run out of recycled PSUM banks against SBUF-held S.  The PE
    never waits on the DVE rope tail.
  - ykv contraction is deferred: S chunks land in SBUF (S_sb/sdiag), then
    8 bank-exclusive PSUM groups accumulate ykv rows.
  - Phase D fuses relu(y_sparse)*x_sparse into one scalar_tensor_tensor where
    profitable, split across DVE/ACT/Pool.
  - LayerNorm chains run fp16 end-to-end (residual stream is fp16), with the
    sqrt on ACT and everything else on DVE.
  - Weight/table DMAs are batched (2 m-tiles or 2-4 rope pairs per DMA) from
    host-prearranged slabs.
Collectives (3 pair + 3 all-reduce) are unchanged from the baseline.

PSUM discipline: every accumulation group owns a full bank ([P,512] f32
tiles even when only half is used) — start=True clears the whole bank.
"""

import math
import sys

for _p in ("/opt/trn_rl_repo",):
    if _p not in sys.path:
        sys.path.insert(0, _p)

import numpy as np

import concourse.bass as bass
import concourse.mybir as mybir
import concourse.tile as tile
from concourse import bacc, bass_utils
from concourse.masks import make_identity

# ---- problem constants (hardcoded per contract) ----
B, T, D, NH, N = 1, 1024, 256, 4, 8192
VOCAB = 256
N_LAYER = 3
EPS = 1e-5
TWO_PI = 2.0 * math.pi
N_CORES = 8
NLOC = N // 2          # latent columns per device: 4096
P = 128
NT = T // P            # 8 t-tiles
KD = D // P            # 2 d-tiles
NM = NLOC // P         # 32 n-tiles per device
NPAIR = NM // 2        # 16 rope pairs
HDT = mybir.dt.float16
F32 = mybir.dt.float32
F8 = mybir.dt.float8e4
YKV_SCALE = 1.0 / 256.0
# scores run in fp8e4 DoubleRow; qr is scaled by 2^QR_EXP via the host
# cos/sin tables and S is descaled by 2^-2*QR_EXP at PSUM copy-out
QR_EXP = 6
S_DESCALE = 2.0 ** (-2 * QR_EXP)

# phase-A emission order: pair-completing (0,16,1,17,...)
EMIT = [(e // 2) + NPAIR * (e % 2) for e in range(NM)]
# scores chunks (c, i): rows t-tile i, cols [base, 512(c+1))
CHUNKS = [(c, i) for c in range(2) for i in range(4 * c + 4)]
G1 = [(0, 0), (0, 1), (0, 2), (0, 3), (1, 0), (1, 1), (1, 2)]
G2 = [(1, 3), (1, 4), (1, 5), (1, 6), (1, 7)]
KSPLIT = 10            # rope pairs in K1
# DoubleRow step r contracts k-tiles (2r, 2r+1); step r (r<8) needs the
# even-part qr of pairs 2r,2r+1, step r+8 their odd parts.  Pair-completion
# order within each split:
R1 = [0, 8, 1, 9, 2, 10, 3, 11, 4, 12]   # ready once pairs 0..9 roped
R2 = [5, 13, 6, 14, 7, 15]               # ready once pairs 10..15 roped
# packed S_sb layout: row i stores global cols [128(i+1), 1024)
SOFF = [0]
for _i in range(NT):
    SOFF.append(SOFF[-1] + (T - P * (_i + 1)))  # total 3584

_CACHE = {}


def _build_program(dbg=False, use_collectives=True, n_layers=N_LAYER):
    def emit_allreduce(nc, groups, ins, outs):
        if use_collectives:
            nc.gpsimd.collective_compute(
                "AllReduce", mybir.AluOpType.add, replica_groups=groups,
                ins=ins, outs=outs)
        else:
            nc.sync.dma_start(outs[0], ins[0])

    nc = bacc.Bacc("TRN2", target_bir_lowering=False, debug=False,
                   num_devices=N_CORES)

    x0_d = nc.dram_tensor("x0", [T, D], HDT, kind="ExternalInput")
    # host slabs, see _host_inputs for layouts
    encw_d = nc.dram_tensor("encw", [P, NM * KD * P], HDT, kind="ExternalInput")
    encvw_d = nc.dram_tensor("encvw", [P, NM * KD * P], HDT,
                             kind="ExternalInput")
    decw_d = nc.dram_tensor("decw", [P, NM * D], HDT, kind="ExternalInput")
    ct_d = nc.dram_tensor("ct", [P, NPAIR * T], HDT, kind="ExternalInput")
    st_d = nc.dram_tensor("st", [P, NPAIR * T], HDT, kind="ExternalInput")
    lmh_d = nc.dram_tensor("lmh", [P, KD * VOCAB], HDT, kind="ExternalInput")
    umask_d = nc.dram_tensor("umask", [P, P], F32, kind="ExternalInput")
    logits_d = nc.dram_tensor("logits", [T, VOCAB], F32, kind="ExternalOutput")

    PAIR_GROUPS = [[0, 1], [2, 3], [4, 5], [6, 7]]
    ALL_GROUP = [list(range(N_CORES))]

    with tile.TileContext(nc) as tc:
        persist = tc.alloc_tile_pool(name="persist", bufs=1)
        dram = tc.alloc_tile_pool(name="dram", bufs=1, space="DRAM")

        # persistent SBUF state
        x_h = persist.tile([P, NT, D], HDT)         # residual stream (natural)
        xT_h = persist.tile([P, KD, T], HDT)        # x^T
        ykvT_h = persist.tile([P, KD, T], HDT)      # ykv_ln^T
        x_sp = persist.tile([P, NM, T], HDT)        # x_sparse^T
        qr8 = persist.tile([P, NPAIR, 2, T], F8)    # roped, DR-interleaved
        S_sb = persist.tile([P, SOFF[NT]], HDT)     # packed strict-upper S
        sdiag = persist.tile([P, NT, P], HDT)       # masked diagonal blocks
        ykv_pre = persist.tile([P, NT, D], HDT)
        ykv_post = persist.tile([P, NT, D], HDT)
        ymlpT_pre = persist.tile([P, KD, T], HDT)
        ymlpT_post = persist.tile([P, KD, T], HDT)
        lmh_sb = persist.tile([P, KD, VOCAB], HDT)
        umask_sb = persist.tile([P, P], F32)
        ident = persist.tile([P, P], HDT)
        eps_sb = persist.tile([P, 1], F32)

        nc.vector.memset(eps_sb[:], float(EPS))
        nc.sync.dma_start(umask_sb[:], umask_d.ap())
        make_identity(nc, ident[:])
        nc.sync.dma_start(
            lmh_sb[:], lmh_d.ap().rearrange("p (k v) -> p k v", k=KD))

        # streaming / working pools
        wenc = tc.alloc_tile_pool(name="wenc", bufs=3)      # enc/encv slabs
        wdec = tc.alloc_tile_pool(name="wdec", bufs=2)
        csp = tc.alloc_tile_pool(name="csp", bufs=2)        # ct/st tiles
        ropep = tc.alloc_tile_pool(name="ropep", bufs=2)
        yxp = tc.alloc_tile_pool(name="yxp", bufs=3)
        lnp = tc.alloc_tile_pool(name="lnp", bufs=4)
        statp = tc.alloc_tile_pool(name="statp", bufs=10)

        engines = {"dve": nc.vector, "act": nc.scalar, "pool": nc.gpsimd}

        def copy_from_psum(eng, dst_ap, src_ap):
            if eng == "act":
                nc.scalar.copy(out=dst_ap, in_=src_ap)
            elif eng == "pool":
                nc.gpsimd.tensor_copy(out=dst_ap, in_=src_ap)
            else:
                nc.vector.tensor_copy(out=dst_ap, in_=src_ap)

        def relu_psum(eng, dst_ap, src_ap):
            if eng == "act":
                nc.scalar.activation(out=dst_ap, in_=src_ap,
                                     func=mybir.ActivationFunctionType.Relu)
            else:
                engines[eng].tensor_scalar_max(dst_ap, src_ap, 0.0)

        def ln_batch(pairs):
            """Stage-major LayerNorm over a batch of (src_ap, out_ap) [P, D]
            tiles — every stage's ops are emitted together so independent
            chains pipeline instead of head-of-line blocking engine queues."""
            n = len(pairs)
            stats = [statp.tile([P, 6], F32, name="ln_stats") for _ in range(n)]
            mvs = [statp.tile([P, 2], F32, name="ln_mv") for _ in range(n)]
            rstds = [statp.tile([P, 1], F32, name="ln_rstd") for _ in range(n)]
            for q, (src, _) in enumerate(pairs):
                nc.vector.bn_stats(out=stats[q][:], in_=src)
            for q in range(n):
                nc.vector.bn_aggr(out=mvs[q][:], in_=stats[q][:])
            for q in range(n):
                nc.scalar.activation(out=rstds[q][:], in_=mvs[q][:, 1:2],
                                     func=mybir.ActivationFunctionType.Sqrt,
                                     bias=eps_sb[:])
            for q in range(n):
                nc.vector.reciprocal(out=rstds[q][:], in_=rstds[q][:])
            for q, (src, out) in enumerate(pairs):
                nc.vector.tensor_scalar(out=out, in0=src,
                                        scalar1=mvs[q][:, 0:1],
                                        scalar2=rstds[q][:],
                                        op0=mybir.AluOpType.subtract,
                                        op1=mybir.AluOpType.mult)

        def layer_norm(src_ap, out_ap):
            ln_batch([(src_ap, out_ap)])

        _tp_cycle = ["dve", "act"]
        _tp_idx = [0]

        def transpose_into(dst_ap, src_ap, pst_pool):
            """PE-transpose a [P, P] fp16 SBUF block into dst (via PSUM)."""
            pst = pst_pool.tile([P, P], HDT, name="pst")
            nc.tensor.transpose(pst[:], src_ap, ident[:])
            eng = _tp_cycle[_tp_idx[0] % 2]
            _tp_idx[0] += 1
            copy_from_psum(eng, dst_ap, pst[:])

        def set_xT_from(j, pst_pool):
            for k in range(KD):
                transpose_into(xT_h[:, k, j * P:(j + 1) * P],
                               x_h[:, j, k * P:(k + 1) * P], pst_pool)

        # ---- initial x = ln(embed[idx]) (gather done on host into x0) ----
        with tc.tile_pool(name="ps_init", bufs=4, space="PSUM") as ps_init:
            for hb in range(2):
                js = range(4 * hb, 4 * hb + 4)
                x0ts = []
                for j in js:
                    x0t = lnp.tile([P, D], HDT, name="x0t", bufs=2)
                    nc.sync.dma_start(x0t[:],
                                      x0_d.ap()[j * P:(j + 1) * P, :])
                    x0ts.append(x0t)
                ln_batch([(x0ts[q][:], x_h[:, j, :])
                          for q, j in enumerate(js)])
                for j in js:
                    set_xT_from(j, ps_init)

        # ---- layers ----
        for layer in range(n_layers):
            # === Phase A: x_sparse^T = relu(enc^T x^T), rope -> qr ===
            with tc.tile_pool(name=f"psA_{layer}", bufs=4,
                              space="PSUM") as psA:
                for e in range(0, NM, 2):
                    # batched weight DMA for emission pair (e, e+1); flat 2D
                    # AP keeps it on the cheap HWDGE path
                    et = wenc.tile([P, 2 * KD * P], HDT, name="enc_t", bufs=2)
                    o0 = e * KD * P
                    nc.sync.dma_start(
                        et[:], encw_d.ap()[:, o0:o0 + 2 * KD * P])
                    pair = e // 2
                    ctt = csp.tile([P, T], HDT, name="ctt", tag="ctt")
                    stt = csp.tile([P, T], HDT, name="stt", tag="stt")
                    nc.sync.dma_start(
                        ctt[:], ct_d.ap()[:, pair * T:(pair + 1) * T])
                    nc.sync.dma_start(
                        stt[:], st_d.ap()[:, pair * T:(pair + 1) * T])
                    for sub in range(2):
                        ei = e + sub
                        m = EMIT[ei]
                        ps = psA.tile([P, T], F32, name="psA")
                        for c in range(2):
                            for k in range(KD):
                                nc.tensor.matmul(
                                    ps[:, c * 512:(c + 1) * 512],
                                    lhsT=et[:, (sub * KD + k) * P:
                                             (sub * KD + k + 1) * P],
                                    rhs=xT_h[:, k, c * 512:(c + 1) * 512],
                                    start=(k == 0), stop=(k == KD - 1))
                        reng = "act" if ei % 4 < 3 else "dve"
                        relu_psum(reng, x_sp[:, m, :], ps[:])
                        if sub == 1:
                            # pair EMIT[e]=p complete -> rope pair p into the
                            # fp8 DoubleRow-interleaved qr8.  Pool only works
                            # SBUF->SBUF, via scalar_tensor_tensor (cheaper
                            # than Pool tensor_tensor in the Q7 model).
                            xe = x_sp[:, pair, :]
                            xo = x_sp[:, pair + NPAIR, :]
                            qe = qr8[:, pair // 2, pair % 2, :]
                            qo = qr8[:, pair // 2 + 8, pair % 2, :]
                            mls = [0, 1, 0, 0] if pair % 2 == 0 else \
                                  [1, 0, 0, 1]
                            cmb = 1 if pair % 4 == 2 else 0

                            def rmul(sel, out, a, b):
                                if sel:
                                    nc.gpsimd.scalar_tensor_tensor(
                                        out, a, 1.0, b,
                                        mybir.AluOpType.mult,
                                        mybir.AluOpType.mult)
                                else:
                                    nc.vector.tensor_mul(out, a, b)

                            def rcmb(sel, out, a, b, op):
                                if sel:
                                    nc.gpsimd.scalar_tensor_tensor(
                                        out, a, 1.0, b,
                                        mybir.AluOpType.mult, op)
                                elif op == mybir.AluOpType.subtract:
                                    nc.vector.tensor_sub(out, a, b)
                                else:
                                    nc.vector.tensor_add(out, a, b)

                            t1 = ropep.tile([P, T], HDT, name="rope_t1",
                                            tag="rope_t", bufs=4)
                            t2 = ropep.tile([P, T], HDT, name="rope_t2",
                                            tag="rope_t", bufs=4)
                            rmul(mls[0], t1[:], xe, ctt[:])
                            rmul(mls[1], t2[:], xo, stt[:])
                            rcmb(cmb, qe, t1[:], t2[:],
                                 mybir.AluOpType.subtract)
                            t3 = ropep.tile([P, T], HDT, name="rope_t1",
                                            tag="rope_t", bufs=4)
                            t4 = ropep.tile([P, T], HDT, name="rope_t2",
                                            tag="rope_t", bufs=4)
                            rmul(mls[2], t3[:], xo, ctt[:])
                            rmul(mls[3], t4[:], xe, stt[:])
                            rcmb(cmb, qo, t3[:], t4[:],
                                 mybir.AluOpType.add)

            # === Phase B: scores (2-pass K-split) + deferred ykv ===
            with tc.tile_pool(name=f"psB_{layer}", bufs=7,
                              space="PSUM") as psB:
                def s_chunk_mm(ps, c, i, rs, start, stop):
                    base = max(c * 512, i * P)
                    width = (c + 1) * 512 - base
                    for n, r_ in enumerate(rs):
                        nc.tensor.matmul(
                            ps[:, :width],
                            lhsT=qr8[:, r_, :, i * P:(i + 1) * P],
                            rhs=qr8[:, r_, :, base:base + width],
                            start=(start and n == 0),
                            stop=(stop and n == len(rs) - 1),
                            perf_mode=mybir.MatmulPerfMode.DoubleRow)

                def s_chunk_out(ps, c, i, idx):
                    base = max(c * 512, i * P)
                    width = (c + 1) * 512 - base
                    diag = (c == i // 4)
                    skip = P if diag else 0  # diag block goes to sdiag only
                    w2 = width - skip
                    if w2 > 0:
                        dst0 = SOFF[i] + (base + skip) - P * (i + 1)
                        if idx % 2 == 0:
                            nc.scalar.mul(out=S_sb[:, dst0:dst0 + w2],
                                          in_=ps[:, skip:width],
                                          mul=S_DESCALE)
                        else:
                            nc.vector.tensor_scalar_mul(
                                S_sb[:, dst0:dst0 + w2],
                                ps[:, skip:width], S_DESCALE)
                    if diag:
                        nc.vector.tensor_mul(sdiag[:, i, :], ps[:, 0:P],
                                             umask_sb[:])

                def ykv_row(j):
                    ykv_ps = psB.tile([P, 512], F32, name=f"ykv_ps{j}", tag="psB")
                    for i2 in range(j + 1):
                        if i2 == j:
                            lhsT = sdiag[:, i2, :]
                        else:
                            o = SOFF[i2] + P * (j - i2 - 1)
                            lhsT = S_sb[:, o:o + P]
                        nc.tensor.matmul(
                            ykv_ps[:, :D], lhsT=lhsT, rhs=x_h[:, i2, :],
                            start=(i2 == 0), stop=(i2 == j))
                    nc.scalar.mul(out=ykv_pre[:, j, :], in_=ykv_ps[:, :D],
                                  mul=YKV_SCALE)

                g1_ps = []
                for (c, i) in G1:
                    ps = psB.tile([P, 512], F32, name=f"psS_{c}_{i}", tag="psB")
                    s_chunk_mm(ps, c, i, R1, start=True, stop=False)
                    g1_ps.append(ps)
                for idx, (c, i) in enumerate(G1):
                    s_chunk_mm(g1_ps[idx], c, i, R2, start=False, stop=True)
                    s_chunk_out(g1_ps[idx], c, i, idx)
                # rows j=0..3 only need c=0 chunks + their diagonals (all G1)
                for j in range(4):
                    ykv_row(j)
                for idx, (c, i) in enumerate(G2):
                    ps = psB.tile([P, 512], F32, name=f"psS_{c}_{i}", tag="psB")
                    s_chunk_mm(ps, c, i, R1 + R2, start=True, stop=True)
                    s_chunk_out(ps, c, i, idx)
                for j in range(4, NT):
                    ykv_row(j)

            # === Phase C: pair AllReduce of ykv, layernorm, transpose ===
            ar_in = dram.tile([T, D], HDT, name=f"arin_{layer}",
                              tag=f"arin_{layer}")
            ar_out = dram.tile([T, D], HDT, name=f"arout_{layer}",
                               tag=f"arout_{layer}")
            nc.gpsimd.dma_start(
                ar_in.rearrange("(j p) d -> p j d", p=P), ykv_pre[:])
            emit_allreduce(nc, PAIR_GROUPS, [ar_in.opt()], [ar_out.opt()])
            with tc.tile_pool(name=f"psT_{layer}", bufs=4,
                              space="PSUM") as psT:
                for hb in range(2):
                    js = list(range(4 * hb, 4 * hb + 4))
                    nc.gpsimd.dma_start(
                        ykv_post[:, js[0]:js[0] + 4, :],
                        ar_out.rearrange("(j p) d -> p j d", p=P)[
                            :, js[0]:js[0] + 4])
                    yls = [lnp.tile([P, D], HDT, name="ykv_ln", bufs=4)
                           for _ in js]
                    ln_batch([(ykv_post[:, j, :], yls[q][:])
                              for q, j in enumerate(js)])
                    for q, j in enumerate(js):
                        for k in range(KD):
                            transpose_into(ykvT_h[:, k, j * P:(j + 1) * P],
                                           yls[q][:, k * P:(k + 1) * P], psT)

            # === Phase D: y_sp = relu(encv^T ykv^T); xy; ymlp^T accum ===
            ar2_in = dram.tile([D, T], HDT, name=f"ar2in_{layer}",
                               tag=f"ar2in_{layer}")
            with tc.tile_pool(name=f"psD_{layer}", bufs=4,
                              space="PSUM") as psD, \
                 tc.tile_pool(name=f"psM_{layer}", bufs=1,
                              space="PSUM") as psM:
                ymlpT_ps = [psM.tile([P, T], F32, name=f"ymlpT_ps{k}",
                                     tag=f"ymlpT_ps{k}") for k in range(KD)]
                for e in range(0, NM, 2):
                    et = wenc.tile([P, 2 * KD * P], HDT, name="encv_t",
                                   bufs=2)
                    o0 = e * KD * P
                    nc.sync.dma_start(
                        et[:], encvw_d.ap()[:, o0:o0 + 2 * KD * P])
                    dwt = wdec.tile([P, 2 * D], HDT, name="dec_t")
                    nc.sync.dma_start(
                        dwt[:], decw_d.ap()[:, e * D:(e + 2) * D])
                    for sub in range(2):
                        m = e + sub
                        xy = yxp.tile([P, T], HDT, name="xy")
                        for c in range(2):
                            # [P,512] half-tiles (1 PSUM bank) -> finer
                            # PE / elementwise pipelining
                            ps = psD.tile([P, 512], F32, name="psD")
                            for k in range(KD):
                                nc.tensor.matmul(
                                    ps[:],
                                    lhsT=et[:, (sub * KD + k) * P:
                                             (sub * KD + k + 1) * P],
                                    rhs=ykvT_h[:, k, c * 512:(c + 1) * 512],
                                    start=(k == 0), stop=(k == KD - 1))
                            xyh = xy[:, c * 512:(c + 1) * 512]
                            xsh = x_sp[:, m, c * 512:(c + 1) * 512]
                            sel = (2 * m + c) % 8
                            if sel < 3:
                                nc.vector.scalar_tensor_tensor(
                                    xyh, ps[:], 0.0, xsh,
                                    mybir.AluOpType.max,
                                    mybir.AluOpType.mult)
                            else:
                                ysp = yxp.tile([P, 512], HDT, name="ysp",
                                               bufs=3)
                                nc.scalar.activation(
                                    out=ysp[:], in_=ps[:],
                                    func=mybir.ActivationFunctionType.Relu)
                                eng = nc.gpsimd if sel in (3, 4) else nc.vector
                                eng.tensor_mul(xyh, xsh, ysp[:])
                            for k in range(KD):
                                nc.tensor.matmul(
                                    ymlpT_ps[k][:, c * 512:(c + 1) * 512],
                                    lhsT=dwt[:, (sub * KD + k) * P:
                                             (sub * KD + k + 1) * P],
                                    rhs=xyh,
                                    start=(m == 0), stop=(m == NM - 1))
                for k in range(KD):
                    nc.scalar.copy(out=ymlpT_pre[:, k, :],
                                   in_=ymlpT_ps[k][:])
            nc.gpsimd.dma_start(
                ar2_in.rearrange("(k p) t -> p k t", p=P), ymlpT_pre[:])

            # === Phase E: 8-way AllReduce of ymlp^T; x = ln(x + ln(ymlp)) ===
            ar2_out = dram.tile([D, T], HDT, name=f"ar2out_{layer}",
                                tag=f"ar2out_{layer}", addr_space="Shared")
            emit_allreduce(nc, ALL_GROUP, [ar2_in.opt()], [ar2_out.opt()])
            with tc.tile_pool(name=f"psE_{layer}", bufs=4,
                              space="PSUM") as psE:
                for hb in range(2):
                    js = list(range(4 * hb, 4 * hb + 4))
                    # t-half back-DMA: cols for these 4 j-tiles
                    nc.gpsimd.dma_start(
                        ymlpT_post[:, :, 512 * hb:512 * hb + 512],
                        ar2_out.rearrange("(k p) t -> p k t", p=P)[
                            :, :, 512 * hb:512 * hb + 512])
                    ymts = [lnp.tile([P, D], HDT, name="ymt", bufs=4)
                            for _ in js]
                    for q, j in enumerate(js):
                        for k in range(KD):
                            transpose_into(ymts[q][:, k * P:(k + 1) * P],
                                           ymlpT_post[:, k,
                                                      j * P:(j + 1) * P],
                                           psE)
                    us = [lnp.tile([P, D], HDT, name="u_ln", bufs=4)
                          for _ in js]
                    ln_batch([(ymts[q][:], us[q][:])
                              for q in range(4)])
                    xns = [lnp.tile([P, D], HDT, name="xn", bufs=4)
                           for _ in js]
                    for q, j in enumerate(js):
                        eng = nc.vector if q % 2 == 0 else nc.gpsimd
                        eng.tensor_add(xns[q][:], x_h[:, j, :], us[q][:])
                    ln_batch([(xns[q][:], x_h[:, j, :])
                              for q, j in enumerate(js)])
                    for j in js:
                        set_xT_from(j, psE)

        # ---- logits = x @ lm_head ----
        with tc.tile_pool(name="psL", bufs=4, space="PSUM") as psL:
            pss, lgs = [], []
            for j in range(NT):
                ps = psL.tile([P, VOCAB], F32, name="psLt")
                for k in range(KD):
                    nc.tensor.matmul(ps[:],
                                     lhsT=xT_h[:, k, j * P:(j + 1) * P],
                                     rhs=lmh_sb[:, k, :],
                                     start=(k == 0), stop=(k == KD - 1))
                pss.append(ps)
                if len(pss) == 4 or j == NT - 1:
                    j0 = j - len(pss) + 1
                    for q, psq in enumerate(pss):
                        lg = lnp.tile([P, VOCAB], HDT, name="lgt", bufs=2)
                        nc.scalar.copy(out=lg[:], in_=psq[:])
                        lgs.append(lg)
                    for q, lg in enumerate(lgs):
                        jq = j0 + q
                        nc.gpsimd.dma_start(
                            logits_d.ap()[jq * P:(jq + 1) * P, :], lg[:])
                    pss, lgs = [], []

        for _pool in (statp, lnp, yxp, ropep, csp, wdec, wenc, dram, persist):
            _pool.release()

    nc.compile()
    return nc


def _host_inputs(idx, embed, encoder, encoder_v, decoder, lm_head):
    """Build the 8 per-core input maps (host-side sharding + slab layouts)."""
    f16 = np.float16
    idx = np.asarray(idx).reshape(-1).astype(np.int64)
    embed = np.asarray(embed, np.float32)
    enc = np.asarray(encoder, np.float32)
    encv = np.asarray(encoder_v, np.float32)
    dec = np.asarray(decoder, np.float32)
    lmh = np.asarray(lm_head, np.float32)

    x0 = embed[idx]  # [T, D] gather on host (pure indexing)

    t = np.arange(0, N, dtype=np.float32)
    q = np.floor(t / 2.0) * 2.0
    freqs = (1.0 / ((2.0 ** 16) ** (q / N)) / TWO_PI).astype(np.float32)
    tvec = np.arange(T, dtype=np.float32)

    umask = (np.arange(P)[:, None] < np.arange(P)[None, :]).astype(np.float32) * S_DESCALE
    lmh_slab = np.ascontiguousarray(
        lmh.reshape(KD, P, VOCAB).transpose(1, 0, 2).reshape(P, KD * VOCAB),
        f16)

    in_maps = []
    for d in range(N_CORES):
        h, half = d // 2, d % 2
        perm = np.concatenate([np.arange(0, NLOC, 2),
                               np.arange(1, NLOC, 2)]) + half * NLOC
        f_loc = freqs[perm[:NLOC // 2]]
        ph = (tvec[None, :] * f_loc[:, None]).astype(np.float32) % 1.0

        # enc slabs [P, m, k, col] with phase-A emission order for encw
        encp = enc[h][:, perm]            # [D, NLOC]
        encvp = encv[h][:, perm]
        # [D, NLOC] -> [k, P(part-of-d), m, col] -> want [p, m, k, col]
        def enc_slab(w, order):
            wr = w.reshape(KD, P, NM, P)      # [k, p, m, col]
            wr = wr.transpose(1, 2, 0, 3)     # [p, m, k, col]
            wr = wr[:, order]
            return np.ascontiguousarray(
                wr.reshape(P, NM * KD * P), f16)

        decp = dec[h * N + perm, :]       # [NLOC, D]
        dec_slab = np.ascontiguousarray(
            decp.reshape(NM, P, D).transpose(1, 0, 2).reshape(P, NM * D), f16)

        ct = np.cos(TWO_PI * ph) * (2.0 ** QR_EXP)   # [NLOC//2, T]
        st = np.sin(TWO_PI * ph) * (2.0 ** QR_EXP)
        ct_slab = np.ascontiguousarray(
            ct.reshape(NPAIR, P, T).transpose(1, 0, 2).reshape(P, NPAIR * T),
            f16)
        st_slab = np.ascontiguousarray(
            st.reshape(NPAIR, P, T).transpose(1, 0, 2).reshape(P, NPAIR * T),
            f16)

        in_maps.append({
            "x0": np.ascontiguousarray(x0, f16),
            "encw": enc_slab(encp, EMIT),
            "encvw": enc_slab(encvp, list(range(NM))),
            "decw": dec_slab,
            "ct": ct_slab,
            "st": st_slab,
            "lmh": lmh_slab,
            "umask": umask,
        })
    return in_maps


def kernel(idx, embed, encoder, encoder_v, decoder, lm_head,
           _trace=False, _tmpdir=None):
    if "nc" not in _CACHE:
        _CACHE["nc"] = _build_program()
    nc = _CACHE["nc"]
    in_maps = _host_inputs(idx, embed, encoder, encoder_v, decoder, lm_head)
    res = bass_utils.run_bass_kernel_spmd(
        nc, in_maps, core_ids=list(range(N_CORES)),
        trace=_trace, tmpdir=_tmpdir)
    _CACHE["last_results"] = res
    logits = res.results[0]["logits"].astype(np.float32).reshape(B, T, VOCAB)
    return logits
